# revision 19
# baseline (speedup 1.0000x reference)
"""Causal single-head attention (B=4, S=2048, D=1024, E=1024) on 8 TRN2 cores.

Sharding: 2 cores per batch. Within a batch, core parity p takes query
blocks {2j+p : j=0..7} (128 rows each) — interleaving balances the causal
triangle so both cores do identical work SHAPES (SPMD-clean) and identical
work AMOUNTS.

K/V projections are pair-split: each core projects K^T/V for its half of
the keys (parity 0 → keys [0,1024), parity 1 → [1024,2048)), then the two
cores of a batch exchange halves with two pipelined 2-core AllGathers
(K^T's issued right after the K projection so it overlaps the V and Q
projections; V's overlaps the Q projection).

All matmuls run on the PE in bf16 (fp32 PSUM accumulate). Host pre-transposes
x to [d, s] layout and pre-casts to bf16 so the kernel needs no on-chip
transposes of x. The causal boundary mask is per-core DATA (additive -1e9
tile applied to the last 256-key chunk of every query block).
"""

import sys

if "/opt/trn_rl_repo" not in sys.path:
    sys.path.insert(0, "/opt/trn_rl_repo")

import numpy as np
import ml_dtypes

B, S, D, E = 4, 2048, 1024, 1024
NCORES = 8
NBLK = 8          # query blocks per core (128 rows each)
CHUNK = 256       # key chunk width for scores
P = 128
SH = S // 2       # keys projected per core
SCALE = 1.0 / 32.0  # 1/sqrt(E)

_cache = {}


class _nullpool:
    def __enter__(self):
        return None

    def __exit__(self, *a):
        return False



def _build_program(reps=1, pair_kv=True, cc_mode="one", psum_bufs=(2, 2, 2), proj_bufs=4, desc=False, wp_bufs=4, dma_t=False):
    import concourse.bass as bass
    import concourse.tile as tile
    from concourse import bacc, mybir
    from concourse.bass import ts, ds
    from contextlib import ExitStack

    dt = mybir.dt
    AF = mybir.ActivationFunctionType

    nc = bacc.Bacc(
        "TRN2", target_bir_lowering=False, debug=False, enable_asserts=False,
        num_devices=NCORES,
    )

    kvw = SH if pair_kv else S
    xt_q = nc.dram_tensor("xt_q", [D, NBLK * P], dt.bfloat16, kind="ExternalInput").ap()
    xt_kv = nc.dram_tensor("xt_kv", [D, kvw], dt.bfloat16, kind="ExternalInput").ap()
    wq = nc.dram_tensor("wq", [D, E], dt.bfloat16, kind="ExternalInput").ap()
    wk = nc.dram_tensor("wk", [D, E], dt.bfloat16, kind="ExternalInput").ap()
    wv = nc.dram_tensor("wv", [D, E], dt.bfloat16, kind="ExternalInput").ap()
    maskd = nc.dram_tensor("mask", [P, CHUNK], dt.float32, kind="ExternalInput").ap()
    identd = nc.dram_tensor("ident", [P, P], dt.bfloat16, kind="ExternalInput").ap()
    out = nc.dram_tensor("out", [NBLK, P, E], dt.float32, kind="ExternalOutput").ap()

    if pair_kv:
        # packed [K^T_half (E x SH) ; V_half (SH x E)] exchange buffers
        cc_in = nc.dram_tensor("cc_in", [2, E, SH], dt.bfloat16).ap()
        cc_out = nc.dram_tensor("cc_out", [4, E, SH], dt.bfloat16).ap()
        cc_outk = nc.dram_tensor("cc_outk", [2, E, SH], dt.bfloat16).ap()
        cc_outv = nc.dram_tensor("cc_outv", [2, SH, E], dt.bfloat16).ap()

    DC = D // P   # 8 contraction chunks
    EC = E // P   # 8 e chunks
    TC = S // P   # 16 key chunks of 128

    with tile.TileContext(nc) as tc, ExitStack() as ctx:
        consts = ctx.enter_context(tc.tile_pool(name="consts", bufs=1))

        wq_sb = consts.tile([P, DC, E], dt.bfloat16, tag="wq")
        wk_sb = consts.tile([P, DC, E], dt.bfloat16, tag="wk")
        wv_sb = consts.tile([P, DC, E], dt.bfloat16, tag="wv")
        xq_sb = consts.tile([P, DC, NBLK * P], dt.bfloat16, tag="xq")
        xkv_sb = consts.tile([P, DC, kvw], dt.bfloat16, tag="xkv")
        qt_sb = consts.tile([P, EC, NBLK * P], dt.bfloat16, tag="qt")
        kt_sb = consts.tile([P, EC, S], dt.bfloat16, tag="kt")
        v_sb = consts.tile([P, TC, E], dt.bfloat16, tag="v")
        mask_sb = consts.tile([P, CHUNK], dt.float32, tag="mask")
        ident_sb = consts.tile([P, P], dt.bfloat16, tag="ident")

        nc.sync.dma_start(wk_sb[:, 0, 0:P], wk[0:P, 0:P])
        nc.sync.dma_start(xkv_sb[:, 0, 0:512], xt_kv[0:P, 0:512])
        nc.sync.dma_start(wk_sb[:, 0, P:E], wk[0:P, P:E])
        nc.sync.dma_start(xkv_sb[:, 0, 512:kvw], xt_kv[0:P, 512:kvw])
        nc.sync.dma_start(mask_sb[:], maskd[:])
        nc.sync.dma_start(ident_sb[:], identd[:])
        for dc in range(1, DC):
            nc.sync.dma_start(wk_sb[:, dc, :], wk[dc * P : (dc + 1) * P, :])
            nc.sync.dma_start(xkv_sb[:, dc, :], xt_kv[dc * P : (dc + 1) * P, :])
        for dc in range(DC):
            nc.sync.dma_start(wv_sb[:, dc, :], wv[dc * P : (dc + 1) * P, :])
        for dc in range(DC):
            nc.sync.dma_start(wq_sb[:, dc, :], wq[dc * P : (dc + 1) * P, :])
            nc.sync.dma_start(xq_sb[:, dc, :], xt_q[dc * P : (dc + 1) * P, :])

        for _rep in range(reps):
            # ---- Phase 1: projections (PE, bf16, accumulate over d in PSUM) ----
            with (
                tc.tile_pool(name="proj_ps", bufs=proj_bufs, space="PSUM") as pp,
                tc.tile_pool(name="stage", bufs=8) as stg,
            ):
                if pair_kv:
                    # K^T half [e, t_local] = Wk^T @ x_kv^T  -> cc_in[0]
                    for ec in range(EC):
                        for tn in range(SH // 512):
                            ps = pp.tile([P, 512], dt.float32, tag="proj")
                            for dc in range(DC):
                                nc.tensor.matmul(
                                    ps[:],
                                    wk_sb[:, dc, ts(ec, P)],
                                    xkv_sb[:, dc, ts(tn, 512)],
                                    start=(dc == 0),
                                    stop=(dc == DC - 1),
                                )
                            st = stg.tile([P, 512], dt.bfloat16, tag="st")
                            nc.any.tensor_copy(st[:], ps[:])
                            nc.gpsimd.dma_start(
                                cc_in[0, ec * P : (ec + 1) * P, ts(tn, 512)], st[:]
                            )
                    if cc_mode == "two":
                        nc.gpsimd.collective_compute(
                            "AllGather",
                            mybir.AluOpType.bypass,
                            replica_groups=[[0, 1], [2, 3], [4, 5], [6, 7]],
                            ins=[cc_in[0]],
                            outs=[cc_outk[:]],
                        )
                    # V half [t_local, e] = x_kv @ Wv  -> cc_in[1]
                    for tcc in range(SH // P):
                        for en in range(E // 512):
                            ps = pp.tile([P, 512], dt.float32, tag="proj")
                            for dc in range(DC):
                                nc.tensor.matmul(
                                    ps[:],
                                    xkv_sb[:, dc, ts(tcc, P)],
                                    wv_sb[:, dc, ts(en, 512)],
                                    start=(dc == 0),
                                    stop=(dc == DC - 1),
                                )
                            st = stg.tile([P, 512], dt.bfloat16, tag="st")
                            nc.any.tensor_copy(st[:], ps[:])
                            nc.gpsimd.dma_start(
                                cc_in[1, tcc * P : (tcc + 1) * P, ts(en, 512)], st[:]
                            )
                    if cc_mode == "two":
                        nc.gpsimd.collective_compute(
                            "AllGather",
                            mybir.AluOpType.bypass,
                            replica_groups=[[0, 1], [2, 3], [4, 5], [6, 7]],
                            ins=[cc_in[1]],
                            outs=[cc_outv[:]],
                        )
                    if cc_mode == "one":
                        nc.gpsimd.collective_compute(
                            "AllGather",
                            mybir.AluOpType.bypass,
                            replica_groups=[[0, 1], [2, 3], [4, 5], [6, 7]],
                            ins=[cc_in[:]],
                            outs=[cc_out[:]],
                        )
                    elif cc_mode == "fake":
                        nc.sync.dma_start(cc_out[0], cc_in[0])
                        nc.sync.dma_start(cc_out[1], cc_in[1])
                        nc.sync.dma_start(cc_out[2], cc_in[0])
                        nc.sync.dma_start(cc_out[3], cc_in[1])
                else:
                    for ec in range(EC):
                        for tn in range(S // 512):
                            ps = pp.tile([P, 512], dt.float32, tag="proj")
                            for dc in range(DC):
                                nc.tensor.matmul(
                                    ps[:],
                                    wk_sb[:, dc, ts(ec, P)],
                                    xkv_sb[:, dc, ts(tn, 512)],
                                    start=(dc == 0),
                                    stop=(dc == DC - 1),
                                )
                            nc.any.tensor_copy(kt_sb[:, ec, ts(tn, 512)], ps[:])
                    for tcc in range(TC):
                        for en in range(E // 512):
                            ps = pp.tile([P, 512], dt.float32, tag="proj")
                            for dc in range(DC):
                                nc.tensor.matmul(
                                    ps[:],
                                    xkv_sb[:, dc, ts(tcc, P)],
                                    wv_sb[:, dc, ts(en, 512)],
                                    start=(dc == 0),
                                    stop=(dc == DC - 1),
                                )
                            nc.any.tensor_copy(v_sb[:, tcc, ts(en, 512)], ps[:])

                # Q^T [e, q] = Wq^T @ x_q^T  (overlaps the collective)
                for ec in range(EC):
                    for qn in range(NBLK * P // 512):
                        ps = pp.tile([P, 512], dt.float32, tag="proj")
                        for dc in range(DC):
                            nc.tensor.matmul(
                                ps[:],
                                wq_sb[:, dc, ts(ec, P)],
                                xq_sb[:, dc, ts(qn, 512)],
                                start=(dc == 0),
                                stop=(dc == DC - 1),
                            )
                        nc.any.tensor_copy(qt_sb[:, ec, ts(qn, 512)], ps[:])

                if pair_kv:
                    # unpack gathered halves into full K^T / V in SBUF
                    for r in range(2):
                        ksrc = cc_outk[r] if cc_mode == "two" else cc_out[2 * r]
                        vsrc = cc_outv[r] if cc_mode == "two" else cc_out[2 * r + 1]
                        for ec in range(EC):
                            nc.gpsimd.dma_start(
                                kt_sb[:, ec, ds(r * SH, SH)],
                                ksrc[ec * P : (ec + 1) * P, :],
                            )
                        for tcl in range(SH // P):
                            nc.gpsimd.dma_start(
                                v_sb[:, r * (SH // P) + tcl, :],
                                vsrc[tcl * P : (tcl + 1) * P, :],
                            )

            # ---- Phase 2: attention ----
            with (
                tc.tile_pool(name="score_ps", bufs=psum_bufs[0], space="PSUM") as sp,
                (tc.tile_pool(name="pt_ps", bufs=psum_bufs[1], space="PSUM")
                 if not dma_t else _nullpool()) as tp,
                tc.tile_pool(name="out_ps", bufs=psum_bufs[2], space="PSUM") as op,
                tc.tile_pool(name="work", bufs=wp_bufs) as wp,
                tc.tile_pool(name="small", bufs=2) as smp,
            ):
                for j in (reversed(range(NBLK)) if desc else range(NBLK)):
                    nch = j + 1  # 256-key chunks for this query block
                    ps_out = op.tile([P, E], dt.float32, tag="ps_out")
                    sums = smp.tile([P, NBLK], dt.float32, tag="sums")
                    for c in range(nch):
                        ps_s = sp.tile([P, CHUNK], dt.float32, tag="ps_s")
                        for ec in range(EC):
                            nc.tensor.matmul(
                                ps_s[:],
                                qt_sb[:, ec, ts(j, P)],
                                kt_sb[:, ec, ds(c * CHUNK, CHUNK)],
                                start=(ec == 0),
                                stop=(ec == EC - 1),
                            )
                        if c == nch - 1:
                            nc.vector.tensor_add(ps_s[:], ps_s[:], mask_sb[:])
                        p_t = wp.tile([P, CHUNK], dt.bfloat16, tag="p")
                        nc.scalar.activation(
                            p_t[:], ps_s[:], AF.Exp,
                            bias=0.0, scale=SCALE,
                            accum_out=sums[:, c : c + 1],
                        )
                        for h in range(2):
                            t_idx = 2 * c + h
                            pt_t = wp.tile([P, P], dt.bfloat16, tag="pt")
                            if dma_t:
                                nc.scalar.dma_start_transpose(pt_t[:], p_t[:, ts(h, P)])
                            else:
                                ps_t = tp.tile([P, P], dt.bfloat16, tag="ps_t")
                                nc.tensor.transpose(ps_t[:], p_t[:, ts(h, P)], ident_sb[:])
                                nc.vector.tensor_copy(pt_t[:], ps_t[:])
                            for en in range(E // 512):
                                nc.tensor.matmul(
                                    ps_out[:, ts(en, 512)],
                                    pt_t[:],
                                    v_sb[:, t_idx, ts(en, 512)],
                                    start=(c == 0 and h == 0),
                                    stop=(c == nch - 1 and h == 1),
                                )
                    denom = smp.tile([P, 1], dt.float32, tag="denom")
                    nc.vector.reduce_sum(
                        denom[:], sums[:, 0:nch], axis=mybir.AxisListType.X
                    )
                    recip = smp.tile([P, 1], dt.float32, tag="recip")
                    nc.vector.reciprocal(recip[:], denom[:])
                    out_t = wp.tile([P, E], dt.float32, tag="out_t")
                    nc.vector.tensor_scalar_mul(out_t[:], ps_out[:], recip[:])
                    nc.gpsimd.dma_start(out[j], out_t[:])

    nc.compile()
    return nc


def _get_program():
    if "nc" not in _cache:
        _cache["nc"] = _build_program(
            reps=1, pair_kv=True, cc_mode="two", psum_bufs=(3, 3, 1), proj_bufs=8
        )
    return _cache["nc"]


def _make_in_maps(x, Wq, Wk, Wv, pair_kv=True):
    bf16 = ml_dtypes.bfloat16
    wq_b = np.ascontiguousarray(Wq.astype(bf16))
    wk_b = np.ascontiguousarray(Wk.astype(bf16))
    wv_b = np.ascontiguousarray(Wv.astype(bf16))

    # additive causal masks for the boundary chunk, per parity
    r = np.arange(P)[:, None]
    c = np.arange(CHUNK)[None, :]
    masks = [
        np.where(c <= r, 0.0, -1e9).astype(np.float32),        # parity 0
        np.where(c <= r + P, 0.0, -1e9).astype(np.float32),    # parity 1
    ]
    ident = np.eye(P, dtype=bf16)

    in_maps = []
    for core in range(NCORES):
        b, par = core // 2, core % 2
        xt = np.ascontiguousarray(x[b].T.astype(bf16))  # [D, S]
        blocks = [2 * j + par for j in range(NBLK)]
        xt_q = np.ascontiguousarray(
            xt.reshape(D, S // P, P)[:, blocks, :].reshape(D, NBLK * P)
        )
        xt_kv = (
            np.ascontiguousarray(xt[:, par * SH : (par + 1) * SH]) if pair_kv else xt
        )
        in_maps.append(
            {
                "xt_q": xt_q,
                "xt_kv": xt_kv,
                "wq": wq_b,
                "wk": wk_b,
                "wv": wv_b,
                "mask": masks[par],
                "ident": ident,
            }
        )
    return in_maps


def _assemble(results):
    out = np.empty((B, S, E), dtype=np.float32)
    for core in range(NCORES):
        b, par = core // 2, core % 2
        o = results[core]["out"]  # [NBLK, P, E]
        for j in range(NBLK):
            i = 2 * j + par
            out[b, i * P : (i + 1) * P, :] = o[j]
    return out


def run(inputs, trace=False):
    from concourse import bass_utils

    x = np.asarray(inputs["x"], dtype=np.float32)
    Wq = np.asarray(inputs["Wq"], dtype=np.float32)
    Wk = np.asarray(inputs["Wk"], dtype=np.float32)
    Wv = np.asarray(inputs["Wv"], dtype=np.float32)

    nc = _get_program()
    in_maps = _make_in_maps(x, Wq, Wk, Wv)
    res = bass_utils.run_bass_kernel_spmd(
        nc, in_maps, core_ids=list(range(NCORES)), trace=trace
    )
    return _assemble(res.results), res


def kernel(**inputs):
    out, _ = run(inputs, trace=False)
    return out


# revision 22
# speedup vs baseline: 1.3127x; 1.3127x over previous
"""Causal single-head attention (B=4, S=2048, D=1024, E=1024) on 8 TRN2 cores.

Sharding: 2 cores per batch. Within a batch, core parity p takes query
blocks {2j+p : j=0..7} (128 rows each) — interleaving balances the causal
triangle so both cores do identical work SHAPES (SPMD-clean) and identical
work AMOUNTS.

K/V projections are pair-split: each core projects K^T/V for its half of
the keys (parity 0 → keys [0,1024), parity 1 → [1024,2048)), then the two
cores of a batch exchange halves with two pipelined 2-core AllGathers
(K^T's issued right after the K projection so it overlaps the V and Q
projections; V's overlaps the Q projection).

All matmuls run on the PE in bf16 (fp32 PSUM accumulate). Host pre-transposes
x to [d, s] layout and pre-casts to bf16 so the kernel needs no on-chip
transposes of x. The causal boundary mask is per-core DATA (additive -1e9
tile applied to the last 256-key chunk of every query block).
"""

import sys

if "/opt/trn_rl_repo" not in sys.path:
    sys.path.insert(0, "/opt/trn_rl_repo")

import numpy as np
import ml_dtypes

B, S, D, E = 4, 2048, 1024, 1024
NCORES = 8
NBLK = 8          # query blocks per core (128 rows each)
CHUNK = 256       # key chunk width for scores
P = 128
SH = S // 2       # keys projected per core
SCALE = 1.0 / 32.0  # 1/sqrt(E)

_cache = {}


class _nullpool:
    def __enter__(self):
        return None

    def __exit__(self, *a):
        return False



def _build_program(reps=1, pair_kv=True, cc_mode="one", psum_bufs=(2, 2, 2), proj_bufs=4, desc=False, wp_bufs=4, dma_t=False, mixed=False):
    import concourse.bass as bass
    import concourse.tile as tile
    from concourse import bacc, mybir
    from concourse.bass import ts, ds
    from contextlib import ExitStack

    dt = mybir.dt
    AF = mybir.ActivationFunctionType

    nc = bacc.Bacc(
        "TRN2", target_bir_lowering=False, debug=False, enable_asserts=False,
        num_devices=NCORES,
    )

    kvw = SH if pair_kv else S
    xt_q = nc.dram_tensor("xt_q", [D, NBLK * P], dt.bfloat16, kind="ExternalInput").ap()
    xt_kv = nc.dram_tensor("xt_kv", [D, kvw], dt.bfloat16, kind="ExternalInput").ap()
    wq = nc.dram_tensor("wq", [D, E], dt.bfloat16, kind="ExternalInput").ap()
    wk = nc.dram_tensor("wk", [D, E], dt.bfloat16, kind="ExternalInput").ap()
    wv = nc.dram_tensor("wv", [D, E], dt.bfloat16, kind="ExternalInput").ap()
    maskd = nc.dram_tensor("mask", [P, CHUNK], dt.float32, kind="ExternalInput").ap()
    identd = nc.dram_tensor("ident", [P, P], dt.bfloat16, kind="ExternalInput").ap()
    out = nc.dram_tensor("out", [NBLK, P, E], dt.float32, kind="ExternalOutput").ap()

    if pair_kv:
        # packed [K^T_half (E x SH) ; V_half (SH x E)] exchange buffers
        cc_in = nc.dram_tensor("cc_in", [2, E, SH], dt.bfloat16).ap()
        cc_out = nc.dram_tensor("cc_out", [4, E, SH], dt.bfloat16).ap()
        cc_outk = nc.dram_tensor("cc_outk", [2, E, SH], dt.bfloat16).ap()
        cc_outv = nc.dram_tensor("cc_outv", [2, SH, E], dt.bfloat16).ap()

    DC = D // P   # 8 contraction chunks
    EC = E // P   # 8 e chunks
    TC = S // P   # 16 key chunks of 128

    with tile.TileContext(nc) as tc, ExitStack() as ctx:
        consts = ctx.enter_context(tc.tile_pool(name="consts", bufs=1))

        wq_sb = consts.tile([P, DC, E], dt.bfloat16, tag="wq")
        wk_sb = consts.tile([P, DC, E], dt.bfloat16, tag="wk")
        wv_sb = consts.tile([P, DC, E], dt.bfloat16, tag="wv")
        xq_sb = consts.tile([P, DC, NBLK * P], dt.bfloat16, tag="xq")
        xkv_sb = consts.tile([P, DC, kvw], dt.bfloat16, tag="xkv")
        qt_sb = consts.tile([P, EC, NBLK * P], dt.bfloat16, tag="qt")
        kt_sb = consts.tile([P, EC, S], dt.bfloat16, tag="kt")
        v_sb = consts.tile([P, TC, E], dt.bfloat16, tag="v")
        mask_sb = consts.tile([P, CHUNK], dt.float32, tag="mask")
        ident_sb = consts.tile([P, P], dt.bfloat16, tag="ident")

        nc.sync.dma_start(wk_sb[:, 0, 0:P], wk[0:P, 0:P])
        nc.sync.dma_start(xkv_sb[:, 0, 0:512], xt_kv[0:P, 0:512])
        nc.sync.dma_start(wk_sb[:, 0, P:E], wk[0:P, P:E])
        nc.sync.dma_start(xkv_sb[:, 0, 512:kvw], xt_kv[0:P, 512:kvw])
        nc.sync.dma_start(mask_sb[:], maskd[:])
        nc.sync.dma_start(ident_sb[:], identd[:])
        for dc in range(1, DC):
            nc.sync.dma_start(wk_sb[:, dc, :], wk[dc * P : (dc + 1) * P, :])
            nc.sync.dma_start(xkv_sb[:, dc, :], xt_kv[dc * P : (dc + 1) * P, :])
        for dc in range(DC):
            nc.sync.dma_start(wv_sb[:, dc, :], wv[dc * P : (dc + 1) * P, :])
        for dc in range(DC):
            nc.sync.dma_start(wq_sb[:, dc, :], wq[dc * P : (dc + 1) * P, :])
            nc.sync.dma_start(xq_sb[:, dc, :], xt_q[dc * P : (dc + 1) * P, :])

        for _rep in range(reps):
            # ---- Phase 1: projections (PE, bf16, accumulate over d in PSUM) ----
            with (
                tc.tile_pool(name="proj_ps", bufs=proj_bufs, space="PSUM") as pp,
                tc.tile_pool(name="stage", bufs=8) as stg,
            ):
                if pair_kv:
                    # K^T half [e, t_local] = Wk^T @ x_kv^T  -> cc_in[0]
                    for ec in range(EC):
                        for tn in range(SH // 512):
                            ps = pp.tile([P, 512], dt.float32, tag="proj")
                            for dc in range(DC):
                                nc.tensor.matmul(
                                    ps[:],
                                    wk_sb[:, dc, ts(ec, P)],
                                    xkv_sb[:, dc, ts(tn, 512)],
                                    start=(dc == 0),
                                    stop=(dc == DC - 1),
                                )
                            st = stg.tile([P, 512], dt.bfloat16, tag="st")
                            nc.any.tensor_copy(st[:], ps[:])
                            nc.gpsimd.dma_start(
                                cc_in[0, ec * P : (ec + 1) * P, ts(tn, 512)], st[:]
                            )
                    if cc_mode == "two":
                        nc.gpsimd.collective_compute(
                            "AllGather",
                            mybir.AluOpType.bypass,
                            replica_groups=[[0, 1], [2, 3], [4, 5], [6, 7]],
                            ins=[cc_in[0]],
                            outs=[cc_outk[:]],
                        )
                    # V half [t_local, e] = x_kv @ Wv  -> cc_in[1]
                    for tcc in range(SH // P):
                        for en in range(E // 512):
                            ps = pp.tile([P, 512], dt.float32, tag="proj")
                            for dc in range(DC):
                                nc.tensor.matmul(
                                    ps[:],
                                    xkv_sb[:, dc, ts(tcc, P)],
                                    wv_sb[:, dc, ts(en, 512)],
                                    start=(dc == 0),
                                    stop=(dc == DC - 1),
                                )
                            st = stg.tile([P, 512], dt.bfloat16, tag="st")
                            nc.any.tensor_copy(st[:], ps[:])
                            nc.gpsimd.dma_start(
                                cc_in[1, tcc * P : (tcc + 1) * P, ts(en, 512)], st[:]
                            )
                    if cc_mode == "two":
                        nc.gpsimd.collective_compute(
                            "AllGather",
                            mybir.AluOpType.bypass,
                            replica_groups=[[0, 1], [2, 3], [4, 5], [6, 7]],
                            ins=[cc_in[1]],
                            outs=[cc_outv[:]],
                        )
                    if cc_mode == "one":
                        nc.gpsimd.collective_compute(
                            "AllGather",
                            mybir.AluOpType.bypass,
                            replica_groups=[[0, 1], [2, 3], [4, 5], [6, 7]],
                            ins=[cc_in[:]],
                            outs=[cc_out[:]],
                        )
                    elif cc_mode == "fake":
                        nc.sync.dma_start(cc_out[0], cc_in[0])
                        nc.sync.dma_start(cc_out[1], cc_in[1])
                        nc.sync.dma_start(cc_out[2], cc_in[0])
                        nc.sync.dma_start(cc_out[3], cc_in[1])
                else:
                    for ec in range(EC):
                        for tn in range(S // 512):
                            ps = pp.tile([P, 512], dt.float32, tag="proj")
                            for dc in range(DC):
                                nc.tensor.matmul(
                                    ps[:],
                                    wk_sb[:, dc, ts(ec, P)],
                                    xkv_sb[:, dc, ts(tn, 512)],
                                    start=(dc == 0),
                                    stop=(dc == DC - 1),
                                )
                            nc.any.tensor_copy(kt_sb[:, ec, ts(tn, 512)], ps[:])
                    for tcc in range(TC):
                        for en in range(E // 512):
                            ps = pp.tile([P, 512], dt.float32, tag="proj")
                            for dc in range(DC):
                                nc.tensor.matmul(
                                    ps[:],
                                    xkv_sb[:, dc, ts(tcc, P)],
                                    wv_sb[:, dc, ts(en, 512)],
                                    start=(dc == 0),
                                    stop=(dc == DC - 1),
                                )
                            nc.any.tensor_copy(v_sb[:, tcc, ts(en, 512)], ps[:])

                # Q^T [e, q] = Wq^T @ x_q^T  (overlaps the collective)
                for ec in range(EC):
                    for qn in range(NBLK * P // 512):
                        ps = pp.tile([P, 512], dt.float32, tag="proj")
                        for dc in range(DC):
                            nc.tensor.matmul(
                                ps[:],
                                wq_sb[:, dc, ts(ec, P)],
                                xq_sb[:, dc, ts(qn, 512)],
                                start=(dc == 0),
                                stop=(dc == DC - 1),
                            )
                        nc.any.tensor_copy(qt_sb[:, ec, ts(qn, 512)], ps[:])

                if pair_kv:
                    # unpack gathered halves into full K^T / V in SBUF
                    for r in range(2):
                        ksrc = cc_outk[r] if cc_mode == "two" else cc_out[2 * r]
                        vsrc = cc_outv[r] if cc_mode == "two" else cc_out[2 * r + 1]
                        for ec in range(EC):
                            nc.gpsimd.dma_start(
                                kt_sb[:, ec, ds(r * SH, SH)],
                                ksrc[ec * P : (ec + 1) * P, :],
                            )
                        for tcl in range(SH // P):
                            nc.gpsimd.dma_start(
                                v_sb[:, r * (SH // P) + tcl, :],
                                vsrc[tcl * P : (tcl + 1) * P, :],
                            )

            # ---- Phase 2: attention ----
            with (
                tc.tile_pool(name="score_ps", bufs=psum_bufs[0], space="PSUM") as sp,
                (tc.tile_pool(name="pt_ps", bufs=psum_bufs[1], space="PSUM")
                 if not dma_t else _nullpool()) as tp,
                tc.tile_pool(name="out_ps", bufs=psum_bufs[2], space="PSUM") as op,
                tc.tile_pool(name="work", bufs=wp_bufs) as wp,
                tc.tile_pool(name="small", bufs=4) as smp,
            ):
                for j in (reversed(range(NBLK)) if desc else range(NBLK)):
                    # chunk plan: (start_key, width, is_boundary); same trip
                    # structure on every core (depends only on slot j)
                    if mixed:
                        chunks = []
                        off = 0
                        for _ in range(j // 2):
                            chunks.append((off, 512, False)); off += 512
                        if j % 2:
                            chunks.append((off, 256, False)); off += 256
                        chunks.append((off, CHUNK, True))
                    else:
                        chunks = [
                            (c * CHUNK, CHUNK, c == j) for c in range(j + 1)
                        ]
                    ps_out = op.tile([P, E], dt.float32, tag="ps_out")
                    sums = smp.tile([P, NBLK], dt.float32, tag="sums")
                    for ci, (start, width, is_b) in enumerate(chunks):
                        ps_s = sp.tile([P, 512 if mixed else CHUNK],
                                       dt.float32, tag="ps_s")
                        for ec in range(EC):
                            nc.tensor.matmul(
                                ps_s[:, 0:width],
                                qt_sb[:, ec, ts(j, P)],
                                kt_sb[:, ec, ds(start, width)],
                                start=(ec == 0),
                                stop=(ec == EC - 1),
                            )
                        if is_b:
                            nc.vector.tensor_add(
                                ps_s[:, 0:width], ps_s[:, 0:width], mask_sb[:]
                            )
                        p_t = wp.tile([P, 512 if mixed else CHUNK],
                                      dt.bfloat16, tag="p")
                        nc.scalar.activation(
                            p_t[:, 0:width], ps_s[:, 0:width], AF.Exp,
                            bias=0.0, scale=SCALE,
                            accum_out=sums[:, ci : ci + 1],
                        )
                        for h in range(width // P):
                            t_idx = start // P + h
                            pt_t = wp.tile([P, P], dt.bfloat16, tag="pt")
                            if dma_t:
                                nc.scalar.dma_start_transpose(pt_t[:], p_t[:, ts(h, P)])
                            else:
                                ps_t = tp.tile([P, P], dt.bfloat16, tag="ps_t")
                                nc.tensor.transpose(ps_t[:], p_t[:, ts(h, P)], ident_sb[:])
                                nc.vector.tensor_copy(pt_t[:], ps_t[:])
                            for en in range(E // 512):
                                nc.tensor.matmul(
                                    ps_out[:, ts(en, 512)],
                                    pt_t[:],
                                    v_sb[:, t_idx, ts(en, 512)],
                                    start=(t_idx == 0),
                                    stop=(is_b and h == width // P - 1),
                                )
                    denom = smp.tile([P, 1], dt.float32, tag="denom")
                    nc.vector.reduce_sum(
                        denom[:], sums[:, 0 : len(chunks)], axis=mybir.AxisListType.X
                    )
                    recip = smp.tile([P, 1], dt.float32, tag="recip")
                    nc.vector.reciprocal(recip[:], denom[:])
                    out_t = wp.tile([P, E], dt.float32, tag="out_t")
                    nc.vector.tensor_scalar_mul(out_t[:], ps_out[:], recip[:])
                    nc.gpsimd.dma_start(out[j], out_t[:])

    nc.compile()
    return nc


def _get_program():
    if "nc" not in _cache:
        _cache["nc"] = _build_program(
            reps=1, pair_kv=True, cc_mode="two", psum_bufs=(3, 3, 1), proj_bufs=8,
            mixed=True,
        )
    return _cache["nc"]


def _make_in_maps(x, Wq, Wk, Wv, pair_kv=True):
    bf16 = ml_dtypes.bfloat16
    wq_b = np.ascontiguousarray(Wq.astype(bf16))
    wk_b = np.ascontiguousarray(Wk.astype(bf16))
    wv_b = np.ascontiguousarray(Wv.astype(bf16))

    # additive causal masks for the boundary chunk, per parity
    r = np.arange(P)[:, None]
    c = np.arange(CHUNK)[None, :]
    masks = [
        np.where(c <= r, 0.0, -1e9).astype(np.float32),        # parity 0
        np.where(c <= r + P, 0.0, -1e9).astype(np.float32),    # parity 1
    ]
    ident = np.eye(P, dtype=bf16)

    in_maps = []
    for core in range(NCORES):
        b, par = core // 2, core % 2
        xt = np.ascontiguousarray(x[b].T.astype(bf16))  # [D, S]
        blocks = [2 * j + par for j in range(NBLK)]
        xt_q = np.ascontiguousarray(
            xt.reshape(D, S // P, P)[:, blocks, :].reshape(D, NBLK * P)
        )
        xt_kv = (
            np.ascontiguousarray(xt[:, par * SH : (par + 1) * SH]) if pair_kv else xt
        )
        in_maps.append(
            {
                "xt_q": xt_q,
                "xt_kv": xt_kv,
                "wq": wq_b,
                "wk": wk_b,
                "wv": wv_b,
                "mask": masks[par],
                "ident": ident,
            }
        )
    return in_maps


def _assemble(results):
    out = np.empty((B, S, E), dtype=np.float32)
    for core in range(NCORES):
        b, par = core // 2, core % 2
        o = results[core]["out"]  # [NBLK, P, E]
        for j in range(NBLK):
            i = 2 * j + par
            out[b, i * P : (i + 1) * P, :] = o[j]
    return out


def run(inputs, trace=False):
    from concourse import bass_utils

    x = np.asarray(inputs["x"], dtype=np.float32)
    Wq = np.asarray(inputs["Wq"], dtype=np.float32)
    Wk = np.asarray(inputs["Wk"], dtype=np.float32)
    Wv = np.asarray(inputs["Wv"], dtype=np.float32)

    nc = _get_program()
    in_maps = _make_in_maps(x, Wq, Wk, Wv)
    res = bass_utils.run_bass_kernel_spmd(
        nc, in_maps, core_ids=list(range(NCORES)), trace=trace
    )
    return _assemble(res.results), res


def kernel(**inputs):
    out, _ = run(inputs, trace=False)
    return out


# revision 25
# speedup vs baseline: 1.6135x; 1.2291x over previous
"""Causal single-head attention (B=4, S=2048, D=1024, E=1024) on 8 TRN2 cores.

Sharding: 2 cores per batch. Within a batch, core parity p takes query
blocks {2j+p : j=0..7} (128 rows each) — interleaving balances the causal
triangle so both cores do identical work SHAPES (SPMD-clean) and identical
work AMOUNTS.

K/V projections are pair-split: each core projects K^T/V for its half of
the keys (parity 0 → keys [0,1024), parity 1 → [1024,2048)), then the two
cores of a batch exchange halves with two pipelined 2-core AllGathers
(K^T's issued right after the K projection so it overlaps the V and Q
projections; V's overlaps the Q projection).

All matmuls run on the PE in bf16 (fp32 PSUM accumulate). Host pre-transposes
x to [d, s] layout and pre-casts to bf16 so the kernel needs no on-chip
transposes of x. The causal boundary mask is per-core DATA (additive -1e9
tile applied to the last 256-key chunk of every query block).
"""

import sys

if "/opt/trn_rl_repo" not in sys.path:
    sys.path.insert(0, "/opt/trn_rl_repo")

import numpy as np
import ml_dtypes

B, S, D, E = 4, 2048, 1024, 1024
NCORES = 8
NBLK = 8          # query blocks per core (128 rows each)
CHUNK = 256       # key chunk width for scores
P = 128
SH = S // 2       # keys projected per core
SCALE = 1.0 / 32.0  # 1/sqrt(E)

_cache = {}


class _nullpool:
    def __enter__(self):
        return None

    def __exit__(self, *a):
        return False



def _build_program(reps=1, pair_kv=True, cc_mode="one", psum_bufs=(2, 2, 2), proj_bufs=4, desc=False, wp_bufs=4, dma_t=False, mixed=False):
    import concourse.bass as bass
    import concourse.tile as tile
    from concourse import bacc, mybir
    from concourse.bass import ts, ds
    from contextlib import ExitStack

    dt = mybir.dt
    AF = mybir.ActivationFunctionType

    nc = bacc.Bacc(
        "TRN2", target_bir_lowering=False, debug=False, enable_asserts=False,
        num_devices=NCORES,
    )

    kvw = SH if pair_kv else S
    xt_q = nc.dram_tensor("xt_q", [D, NBLK * P], dt.bfloat16, kind="ExternalInput").ap()
    xt_kv = nc.dram_tensor("xt_kv", [D, kvw], dt.bfloat16, kind="ExternalInput").ap()
    wq = nc.dram_tensor("wq", [D, E], dt.bfloat16, kind="ExternalInput").ap()
    wk = nc.dram_tensor("wk", [D, E], dt.bfloat16, kind="ExternalInput").ap()
    wv = nc.dram_tensor("wv", [D, E], dt.bfloat16, kind="ExternalInput").ap()
    maskd = nc.dram_tensor("mask", [P, CHUNK], dt.float32, kind="ExternalInput").ap()
    identd = nc.dram_tensor("ident", [P, P], dt.bfloat16, kind="ExternalInput").ap()
    out = nc.dram_tensor("out", [NBLK, P, E], dt.float32, kind="ExternalOutput").ap()

    if pair_kv:
        # packed [K^T_half (E x SH) ; V_half (SH x E)] exchange buffers
        cc_in = nc.dram_tensor("cc_in", [2, E, SH], dt.bfloat16).ap()
        cc_out = nc.dram_tensor("cc_out", [4, E, SH], dt.bfloat16).ap()
        cc_outk = nc.dram_tensor("cc_outk", [2, E, SH], dt.bfloat16).ap()
        cc_outv = nc.dram_tensor("cc_outv", [2, SH, E], dt.bfloat16).ap()

    DC = D // P   # 8 contraction chunks
    EC = E // P   # 8 e chunks
    TC = S // P   # 16 key chunks of 128

    with tile.TileContext(nc) as tc, ExitStack() as ctx:
        consts = ctx.enter_context(tc.tile_pool(name="consts", bufs=1))

        wq_sb = consts.tile([P, DC, E], dt.bfloat16, tag="wq")
        wk_sb = consts.tile([P, DC, E], dt.bfloat16, tag="wk")
        wv_sb = consts.tile([P, DC, E], dt.bfloat16, tag="wv")
        xq_sb = consts.tile([P, DC, NBLK * P], dt.bfloat16, tag="xq")
        xkv_sb = consts.tile([P, DC, kvw], dt.bfloat16, tag="xkv")
        qt_sb = consts.tile([P, EC, NBLK * P], dt.bfloat16, tag="qt")
        kt_sb = consts.tile([P, EC, S], dt.bfloat16, tag="kt")
        v_sb = consts.tile([P, TC, E], dt.bfloat16, tag="v")
        mask_sb = consts.tile([P, CHUNK], dt.float32, tag="mask")
        ident_sb = consts.tile([P, P], dt.bfloat16, tag="ident")

        nc.sync.dma_start(wk_sb[:, 0, 0:P], wk[0:P, 0:P])
        nc.sync.dma_start(xkv_sb[:, 0, 0:512], xt_kv[0:P, 0:512])
        nc.sync.dma_start(wk_sb[:, 0, P:E], wk[0:P, P:E])
        nc.sync.dma_start(xkv_sb[:, 0, 512:kvw], xt_kv[0:P, 512:kvw])
        nc.sync.dma_start(mask_sb[:], maskd[:])
        nc.sync.dma_start(ident_sb[:], identd[:])
        for dc in range(1, DC):
            nc.sync.dma_start(wk_sb[:, dc, :], wk[dc * P : (dc + 1) * P, :])
            nc.sync.dma_start(xkv_sb[:, dc, :], xt_kv[dc * P : (dc + 1) * P, :])
        for dc in range(DC):
            nc.sync.dma_start(wv_sb[:, dc, :], wv[dc * P : (dc + 1) * P, :])
        for dc in range(DC):
            nc.sync.dma_start(wq_sb[:, dc, :], wq[dc * P : (dc + 1) * P, :])
            nc.sync.dma_start(xq_sb[:, dc, :], xt_q[dc * P : (dc + 1) * P, :])

        for _rep in range(reps):
            # ---- Phase 1: projections (PE, bf16, accumulate over d in PSUM) ----
            with (
                tc.tile_pool(name="proj_ps", bufs=proj_bufs, space="PSUM") as pp,
                tc.tile_pool(name="stage", bufs=8) as stg,
            ):
                if pair_kv:
                    # K^T half [e, t_local] = Wk^T @ x_kv^T  -> cc_in[0]
                    for ec in range(EC):
                        for tn in range(SH // 512):
                            ps = pp.tile([P, 512], dt.float32, tag="proj")
                            for dc in range(DC):
                                nc.tensor.matmul(
                                    ps[:],
                                    wk_sb[:, dc, ts(ec, P)],
                                    xkv_sb[:, dc, ts(tn, 512)],
                                    start=(dc == 0),
                                    stop=(dc == DC - 1),
                                )
                            st = stg.tile([P, 512], dt.bfloat16, tag="st")
                            nc.any.tensor_copy(st[:], ps[:])
                            nc.gpsimd.dma_start(
                                cc_in[0, ec * P : (ec + 1) * P, ts(tn, 512)], st[:]
                            )
                    if cc_mode == "two":
                        nc.gpsimd.collective_compute(
                            "AllGather",
                            mybir.AluOpType.bypass,
                            replica_groups=[[0, 1], [2, 3], [4, 5], [6, 7]],
                            ins=[cc_in[0]],
                            outs=[cc_outk[:]],
                        )
                    # V half [t_local, e] = x_kv @ Wv  -> cc_in[1]
                    for tcc in range(SH // P):
                        for en in range(E // 512):
                            ps = pp.tile([P, 512], dt.float32, tag="proj")
                            for dc in range(DC):
                                nc.tensor.matmul(
                                    ps[:],
                                    xkv_sb[:, dc, ts(tcc, P)],
                                    wv_sb[:, dc, ts(en, 512)],
                                    start=(dc == 0),
                                    stop=(dc == DC - 1),
                                )
                            st = stg.tile([P, 512], dt.bfloat16, tag="st")
                            nc.any.tensor_copy(st[:], ps[:])
                            nc.gpsimd.dma_start(
                                cc_in[1, tcc * P : (tcc + 1) * P, ts(en, 512)], st[:]
                            )
                    if cc_mode == "two":
                        nc.gpsimd.collective_compute(
                            "AllGather",
                            mybir.AluOpType.bypass,
                            replica_groups=[[0, 1], [2, 3], [4, 5], [6, 7]],
                            ins=[cc_in[1]],
                            outs=[cc_outv[:]],
                        )
                    if cc_mode == "one":
                        nc.gpsimd.collective_compute(
                            "AllGather",
                            mybir.AluOpType.bypass,
                            replica_groups=[[0, 1], [2, 3], [4, 5], [6, 7]],
                            ins=[cc_in[:]],
                            outs=[cc_out[:]],
                        )
                    elif cc_mode == "fake":
                        nc.sync.dma_start(cc_out[0], cc_in[0])
                        nc.sync.dma_start(cc_out[1], cc_in[1])
                        nc.sync.dma_start(cc_out[2], cc_in[0])
                        nc.sync.dma_start(cc_out[3], cc_in[1])
                else:
                    for ec in range(EC):
                        for tn in range(S // 512):
                            ps = pp.tile([P, 512], dt.float32, tag="proj")
                            for dc in range(DC):
                                nc.tensor.matmul(
                                    ps[:],
                                    wk_sb[:, dc, ts(ec, P)],
                                    xkv_sb[:, dc, ts(tn, 512)],
                                    start=(dc == 0),
                                    stop=(dc == DC - 1),
                                )
                            nc.any.tensor_copy(kt_sb[:, ec, ts(tn, 512)], ps[:])
                    for tcc in range(TC):
                        for en in range(E // 512):
                            ps = pp.tile([P, 512], dt.float32, tag="proj")
                            for dc in range(DC):
                                nc.tensor.matmul(
                                    ps[:],
                                    xkv_sb[:, dc, ts(tcc, P)],
                                    wv_sb[:, dc, ts(en, 512)],
                                    start=(dc == 0),
                                    stop=(dc == DC - 1),
                                )
                            nc.any.tensor_copy(v_sb[:, tcc, ts(en, 512)], ps[:])

                # Q^T [e, q] = Wq^T @ x_q^T  (overlaps the collective)
                for ec in range(EC):
                    for qn in range(NBLK * P // 512):
                        ps = pp.tile([P, 512], dt.float32, tag="proj")
                        for dc in range(DC):
                            nc.tensor.matmul(
                                ps[:],
                                wq_sb[:, dc, ts(ec, P)],
                                xq_sb[:, dc, ts(qn, 512)],
                                start=(dc == 0),
                                stop=(dc == DC - 1),
                            )
                        nc.any.tensor_copy(qt_sb[:, ec, ts(qn, 512)], ps[:])

                if pair_kv:
                    # unpack gathered halves into full K^T / V in SBUF
                    for r in range(2):
                        ksrc = cc_outk[r] if cc_mode == "two" else cc_out[2 * r]
                        vsrc = cc_outv[r] if cc_mode == "two" else cc_out[2 * r + 1]
                        for ec in range(EC):
                            nc.gpsimd.dma_start(
                                kt_sb[:, ec, ds(r * SH, SH)],
                                ksrc[ec * P : (ec + 1) * P, :],
                            )
                        for tcl in range(SH // P):
                            nc.gpsimd.dma_start(
                                v_sb[:, r * (SH // P) + tcl, :],
                                vsrc[tcl * P : (tcl + 1) * P, :],
                            )

            # ---- Phase 2: attention ----
            with (
                tc.tile_pool(name="score_ps", bufs=psum_bufs[0], space="PSUM") as sp,
                (tc.tile_pool(name="pt_ps", bufs=psum_bufs[1], space="PSUM")
                 if not dma_t else _nullpool()) as tp,
                tc.tile_pool(name="out_ps", bufs=psum_bufs[2], space="PSUM") as op,
                tc.tile_pool(name="work", bufs=wp_bufs) as wp,
                tc.tile_pool(name="small", bufs=4) as smp,
            ):
                for j in (reversed(range(NBLK)) if desc else range(NBLK)):
                    # chunk plan: (start_key, width, is_boundary); same trip
                    # structure on every core (depends only on slot j)
                    if mixed:
                        chunks = []
                        off = 0
                        for _ in range(j // 2):
                            chunks.append((off, 512, False)); off += 512
                        if j % 2:
                            chunks.append((off, 256, False)); off += 256
                        chunks.append((off, CHUNK, True))
                    else:
                        chunks = [
                            (c * CHUNK, CHUNK, c == j) for c in range(j + 1)
                        ]
                    ps_out = op.tile([P, E], dt.float32, tag="ps_out")
                    sums = smp.tile([P, NBLK], dt.float32, tag="sums")
                    for ci, (start, width, is_b) in enumerate(chunks):
                        ps_s = sp.tile([P, 512 if mixed else CHUNK],
                                       dt.float32, tag="ps_s")
                        for ec in range(EC):
                            nc.tensor.matmul(
                                ps_s[:, 0:width],
                                qt_sb[:, ec, ts(j, P)],
                                kt_sb[:, ec, ds(start, width)],
                                start=(ec == 0),
                                stop=(ec == EC - 1),
                            )
                        if is_b:
                            nc.vector.tensor_add(
                                ps_s[:, 0:width], ps_s[:, 0:width], mask_sb[:]
                            )
                        p_t = wp.tile([P, 512 if mixed else CHUNK],
                                      dt.bfloat16, tag="p")
                        nc.scalar.activation(
                            p_t[:, 0:width], ps_s[:, 0:width], AF.Exp,
                            bias=0.0, scale=SCALE,
                            accum_out=sums[:, ci : ci + 1],
                        )
                        for h in range(width // P):
                            t_idx = start // P + h
                            pt_t = wp.tile([P, P], dt.bfloat16, tag="pt")
                            if dma_t:
                                nc.scalar.dma_start_transpose(pt_t[:], p_t[:, ts(h, P)])
                            else:
                                ps_t = tp.tile([P, P], dt.bfloat16, tag="ps_t")
                                nc.tensor.transpose(ps_t[:], p_t[:, ts(h, P)], ident_sb[:])
                                nc.vector.tensor_copy(pt_t[:], ps_t[:])
                            for en in range(E // 512):
                                nc.tensor.matmul(
                                    ps_out[:, ts(en, 512)],
                                    pt_t[:],
                                    v_sb[:, t_idx, ts(en, 512)],
                                    start=(t_idx == 0),
                                    stop=(is_b and h == width // P - 1),
                                )
                    denom = smp.tile([P, 1], dt.float32, tag="denom")
                    nc.vector.reduce_sum(
                        denom[:], sums[:, 0 : len(chunks)], axis=mybir.AxisListType.X
                    )
                    recip = smp.tile([P, 1], dt.float32, tag="recip")
                    nc.vector.reciprocal(recip[:], denom[:])
                    out_t = wp.tile([P, E], dt.float32, tag="out_t")
                    for en in range(E // 512):
                        nc.vector.tensor_scalar_mul(
                            out_t[:, ts(en, 512)], ps_out[:, ts(en, 512)], recip[:]
                        )
                        nc.gpsimd.dma_start(
                            out[j][:, ts(en, 512)], out_t[:, ts(en, 512)]
                        )

    nc.compile()
    return nc


def _get_program():
    if "nc" not in _cache:
        _cache["nc"] = _build_program(
            reps=1, pair_kv=True, cc_mode="two", psum_bufs=(3, 3, 1), proj_bufs=8,
            mixed=True,
        )
    return _cache["nc"]


def _make_in_maps(x, Wq, Wk, Wv, pair_kv=True):
    bf16 = ml_dtypes.bfloat16
    wq_b = np.ascontiguousarray(Wq.astype(bf16))
    wk_b = np.ascontiguousarray(Wk.astype(bf16))
    wv_b = np.ascontiguousarray(Wv.astype(bf16))

    # additive causal masks for the boundary chunk, per parity
    r = np.arange(P)[:, None]
    c = np.arange(CHUNK)[None, :]
    masks = [
        np.where(c <= r, 0.0, -1e9).astype(np.float32),        # parity 0
        np.where(c <= r + P, 0.0, -1e9).astype(np.float32),    # parity 1
    ]
    ident = np.eye(P, dtype=bf16)

    in_maps = []
    for core in range(NCORES):
        b, par = core // 2, core % 2
        xt = np.ascontiguousarray(x[b].T.astype(bf16))  # [D, S]
        blocks = [2 * j + par for j in range(NBLK)]
        xt_q = np.ascontiguousarray(
            xt.reshape(D, S // P, P)[:, blocks, :].reshape(D, NBLK * P)
        )
        xt_kv = (
            np.ascontiguousarray(xt[:, par * SH : (par + 1) * SH]) if pair_kv else xt
        )
        in_maps.append(
            {
                "xt_q": xt_q,
                "xt_kv": xt_kv,
                "wq": wq_b,
                "wk": wk_b,
                "wv": wv_b,
                "mask": masks[par],
                "ident": ident,
            }
        )
    return in_maps


def _assemble(results):
    out = np.empty((B, S, E), dtype=np.float32)
    for core in range(NCORES):
        b, par = core // 2, core % 2
        o = results[core]["out"]  # [NBLK, P, E]
        for j in range(NBLK):
            i = 2 * j + par
            out[b, i * P : (i + 1) * P, :] = o[j]
    return out


def run(inputs, trace=False):
    from concourse import bass_utils

    x = np.asarray(inputs["x"], dtype=np.float32)
    Wq = np.asarray(inputs["Wq"], dtype=np.float32)
    Wk = np.asarray(inputs["Wk"], dtype=np.float32)
    Wv = np.asarray(inputs["Wv"], dtype=np.float32)

    nc = _get_program()
    in_maps = _make_in_maps(x, Wq, Wk, Wv)
    res = bass_utils.run_bass_kernel_spmd(
        nc, in_maps, core_ids=list(range(NCORES)), trace=trace
    )
    return _assemble(res.results), res


def kernel(**inputs):
    out, _ = run(inputs, trace=False)
    return out


# revision 26
# speedup vs baseline: 1.6401x; 1.0165x over previous
"""Causal single-head attention (B=4, S=2048, D=1024, E=1024) on 8 TRN2 cores.

Sharding: 2 cores per batch. Within a batch, core parity p takes query
blocks {2j+p : j=0..7} (128 rows each) — interleaving balances the causal
triangle so both cores do identical work SHAPES (SPMD-clean) and identical
work AMOUNTS.

K/V projections are pair-split: each core projects K^T/V for its half of
the keys (parity 0 → keys [0,1024), parity 1 → [1024,2048)), then the two
cores of a batch exchange halves with two pipelined 2-core AllGathers
(K^T's issued right after the K projection so it overlaps the V and Q
projections; V's overlaps the Q projection).

All matmuls run on the PE in bf16 (fp32 PSUM accumulate). Host pre-transposes
x to [d, s] layout and pre-casts to bf16 so the kernel needs no on-chip
transposes of x. The causal boundary mask is per-core DATA (additive -1e9
tile applied to the last 256-key chunk of every query block).
"""

import sys

if "/opt/trn_rl_repo" not in sys.path:
    sys.path.insert(0, "/opt/trn_rl_repo")

import numpy as np
import ml_dtypes

B, S, D, E = 4, 2048, 1024, 1024
NCORES = 8
NBLK = 8          # query blocks per core (128 rows each)
CHUNK = 256       # key chunk width for scores
P = 128
SH = S // 2       # keys projected per core
SCALE = 1.0 / 32.0  # 1/sqrt(E)

_cache = {}


class _nullpool:
    def __enter__(self):
        return None

    def __exit__(self, *a):
        return False



def _build_program(reps=1, pair_kv=True, cc_mode="one", psum_bufs=(2, 2, 2), proj_bufs=4, desc=False, wp_bufs=4, dma_t=False, mixed=False, delay_epi=False):
    import concourse.bass as bass
    import concourse.tile as tile
    from concourse import bacc, mybir
    from concourse.bass import ts, ds
    from contextlib import ExitStack

    dt = mybir.dt
    AF = mybir.ActivationFunctionType

    nc = bacc.Bacc(
        "TRN2", target_bir_lowering=False, debug=False, enable_asserts=False,
        num_devices=NCORES,
    )

    kvw = SH if pair_kv else S
    xt_q = nc.dram_tensor("xt_q", [D, NBLK * P], dt.bfloat16, kind="ExternalInput").ap()
    xt_kv = nc.dram_tensor("xt_kv", [D, kvw], dt.bfloat16, kind="ExternalInput").ap()
    wq = nc.dram_tensor("wq", [D, E], dt.bfloat16, kind="ExternalInput").ap()
    wk = nc.dram_tensor("wk", [D, E], dt.bfloat16, kind="ExternalInput").ap()
    wv = nc.dram_tensor("wv", [D, E], dt.bfloat16, kind="ExternalInput").ap()
    maskd = nc.dram_tensor("mask", [P, CHUNK], dt.float32, kind="ExternalInput").ap()
    identd = nc.dram_tensor("ident", [P, P], dt.bfloat16, kind="ExternalInput").ap()
    out = nc.dram_tensor("out", [NBLK, P, E], dt.float32, kind="ExternalOutput").ap()

    if pair_kv:
        # packed [K^T_half (E x SH) ; V_half (SH x E)] exchange buffers
        cc_in = nc.dram_tensor("cc_in", [2, E, SH], dt.bfloat16).ap()
        cc_out = nc.dram_tensor("cc_out", [4, E, SH], dt.bfloat16).ap()
        cc_outk = nc.dram_tensor("cc_outk", [2, E, SH], dt.bfloat16).ap()
        cc_outv = nc.dram_tensor("cc_outv", [2, SH, E], dt.bfloat16).ap()

    DC = D // P   # 8 contraction chunks
    EC = E // P   # 8 e chunks
    TC = S // P   # 16 key chunks of 128

    with tile.TileContext(nc) as tc, ExitStack() as ctx:
        consts = ctx.enter_context(tc.tile_pool(name="consts", bufs=1))

        wq_sb = consts.tile([P, DC, E], dt.bfloat16, tag="wq")
        wk_sb = consts.tile([P, DC, E], dt.bfloat16, tag="wk")
        wv_sb = consts.tile([P, DC, E], dt.bfloat16, tag="wv")
        xq_sb = consts.tile([P, DC, NBLK * P], dt.bfloat16, tag="xq")
        xkv_sb = consts.tile([P, DC, kvw], dt.bfloat16, tag="xkv")
        qt_sb = consts.tile([P, EC, NBLK * P], dt.bfloat16, tag="qt")
        kt_sb = consts.tile([P, EC, S], dt.bfloat16, tag="kt")
        v_sb = consts.tile([P, TC, E], dt.bfloat16, tag="v")
        mask_sb = consts.tile([P, CHUNK], dt.float32, tag="mask")
        ident_sb = consts.tile([P, P], dt.bfloat16, tag="ident")

        nc.sync.dma_start(wk_sb[:, 0, 0:P], wk[0:P, 0:P])
        nc.sync.dma_start(xkv_sb[:, 0, 0:512], xt_kv[0:P, 0:512])
        nc.sync.dma_start(wk_sb[:, 0, P:E], wk[0:P, P:E])
        nc.sync.dma_start(xkv_sb[:, 0, 512:kvw], xt_kv[0:P, 512:kvw])
        nc.sync.dma_start(mask_sb[:], maskd[:])
        nc.sync.dma_start(ident_sb[:], identd[:])
        for dc in range(1, DC):
            nc.sync.dma_start(wk_sb[:, dc, :], wk[dc * P : (dc + 1) * P, :])
            nc.sync.dma_start(xkv_sb[:, dc, :], xt_kv[dc * P : (dc + 1) * P, :])
        for dc in range(DC):
            nc.sync.dma_start(wv_sb[:, dc, :], wv[dc * P : (dc + 1) * P, :])
        for dc in range(DC):
            nc.sync.dma_start(wq_sb[:, dc, :], wq[dc * P : (dc + 1) * P, :])
            nc.sync.dma_start(xq_sb[:, dc, :], xt_q[dc * P : (dc + 1) * P, :])

        for _rep in range(reps):
            # ---- Phase 1: projections (PE, bf16, accumulate over d in PSUM) ----
            with (
                tc.tile_pool(name="proj_ps", bufs=proj_bufs, space="PSUM") as pp,
                tc.tile_pool(name="stage", bufs=8) as stg,
            ):
                if pair_kv:
                    # K^T half [e, t_local] = Wk^T @ x_kv^T  -> cc_in[0]
                    for ec in range(EC):
                        for tn in range(SH // 512):
                            ps = pp.tile([P, 512], dt.float32, tag="proj")
                            for dc in range(DC):
                                nc.tensor.matmul(
                                    ps[:],
                                    wk_sb[:, dc, ts(ec, P)],
                                    xkv_sb[:, dc, ts(tn, 512)],
                                    start=(dc == 0),
                                    stop=(dc == DC - 1),
                                )
                            st = stg.tile([P, 512], dt.bfloat16, tag="st")
                            nc.any.tensor_copy(st[:], ps[:])
                            nc.gpsimd.dma_start(
                                cc_in[0, ec * P : (ec + 1) * P, ts(tn, 512)], st[:]
                            )
                    if cc_mode == "two":
                        nc.gpsimd.collective_compute(
                            "AllGather",
                            mybir.AluOpType.bypass,
                            replica_groups=[[0, 1], [2, 3], [4, 5], [6, 7]],
                            ins=[cc_in[0]],
                            outs=[cc_outk[:]],
                        )
                    # V half [t_local, e] = x_kv @ Wv  -> cc_in[1]
                    for tcc in range(SH // P):
                        for en in range(E // 512):
                            ps = pp.tile([P, 512], dt.float32, tag="proj")
                            for dc in range(DC):
                                nc.tensor.matmul(
                                    ps[:],
                                    xkv_sb[:, dc, ts(tcc, P)],
                                    wv_sb[:, dc, ts(en, 512)],
                                    start=(dc == 0),
                                    stop=(dc == DC - 1),
                                )
                            st = stg.tile([P, 512], dt.bfloat16, tag="st")
                            nc.any.tensor_copy(st[:], ps[:])
                            nc.gpsimd.dma_start(
                                cc_in[1, tcc * P : (tcc + 1) * P, ts(en, 512)], st[:]
                            )
                    if cc_mode == "two":
                        nc.gpsimd.collective_compute(
                            "AllGather",
                            mybir.AluOpType.bypass,
                            replica_groups=[[0, 1], [2, 3], [4, 5], [6, 7]],
                            ins=[cc_in[1]],
                            outs=[cc_outv[:]],
                        )
                    if cc_mode == "one":
                        nc.gpsimd.collective_compute(
                            "AllGather",
                            mybir.AluOpType.bypass,
                            replica_groups=[[0, 1], [2, 3], [4, 5], [6, 7]],
                            ins=[cc_in[:]],
                            outs=[cc_out[:]],
                        )
                    elif cc_mode == "fake":
                        nc.sync.dma_start(cc_out[0], cc_in[0])
                        nc.sync.dma_start(cc_out[1], cc_in[1])
                        nc.sync.dma_start(cc_out[2], cc_in[0])
                        nc.sync.dma_start(cc_out[3], cc_in[1])
                else:
                    for ec in range(EC):
                        for tn in range(S // 512):
                            ps = pp.tile([P, 512], dt.float32, tag="proj")
                            for dc in range(DC):
                                nc.tensor.matmul(
                                    ps[:],
                                    wk_sb[:, dc, ts(ec, P)],
                                    xkv_sb[:, dc, ts(tn, 512)],
                                    start=(dc == 0),
                                    stop=(dc == DC - 1),
                                )
                            nc.any.tensor_copy(kt_sb[:, ec, ts(tn, 512)], ps[:])
                    for tcc in range(TC):
                        for en in range(E // 512):
                            ps = pp.tile([P, 512], dt.float32, tag="proj")
                            for dc in range(DC):
                                nc.tensor.matmul(
                                    ps[:],
                                    xkv_sb[:, dc, ts(tcc, P)],
                                    wv_sb[:, dc, ts(en, 512)],
                                    start=(dc == 0),
                                    stop=(dc == DC - 1),
                                )
                            nc.any.tensor_copy(v_sb[:, tcc, ts(en, 512)], ps[:])

                # Q^T [e, q] = Wq^T @ x_q^T  (overlaps the collective)
                for ec in range(EC):
                    for qn in range(NBLK * P // 512):
                        ps = pp.tile([P, 512], dt.float32, tag="proj")
                        for dc in range(DC):
                            nc.tensor.matmul(
                                ps[:],
                                wq_sb[:, dc, ts(ec, P)],
                                xq_sb[:, dc, ts(qn, 512)],
                                start=(dc == 0),
                                stop=(dc == DC - 1),
                            )
                        nc.any.tensor_copy(qt_sb[:, ec, ts(qn, 512)], ps[:])

                if pair_kv:
                    # unpack gathered halves into full K^T / V in SBUF
                    for r in range(2):
                        ksrc = cc_outk[r] if cc_mode == "two" else cc_out[2 * r]
                        vsrc = cc_outv[r] if cc_mode == "two" else cc_out[2 * r + 1]
                        for ec in range(EC):
                            nc.gpsimd.dma_start(
                                kt_sb[:, ec, ds(r * SH, SH)],
                                ksrc[ec * P : (ec + 1) * P, :],
                            )
                        for tcl in range(SH // P):
                            nc.gpsimd.dma_start(
                                v_sb[:, r * (SH // P) + tcl, :],
                                vsrc[tcl * P : (tcl + 1) * P, :],
                            )

            # ---- Phase 2: attention ----
            with (
                tc.tile_pool(name="score_ps", bufs=psum_bufs[0], space="PSUM") as sp,
                (tc.tile_pool(name="pt_ps", bufs=psum_bufs[1], space="PSUM")
                 if not dma_t else _nullpool()) as tp,
                tc.tile_pool(name="out_ps", bufs=psum_bufs[2], space="PSUM") as op,
                tc.tile_pool(name="work", bufs=wp_bufs) as wp,
                tc.tile_pool(name="small", bufs=4) as smp,
            ):
                pending_epi = None
                for j in (reversed(range(NBLK)) if desc else range(NBLK)):
                    # chunk plan: (start_key, width, is_boundary); same trip
                    # structure on every core (depends only on slot j)
                    if mixed:
                        chunks = []
                        off = 0
                        for _ in range(j // 2):
                            chunks.append((off, 512, False)); off += 512
                        if j % 2:
                            chunks.append((off, 256, False)); off += 256
                        chunks.append((off, CHUNK, True))
                    else:
                        chunks = [
                            (c * CHUNK, CHUNK, c == j) for c in range(j + 1)
                        ]
                    ps_out = op.tile([P, E], dt.float32, tag="ps_out")
                    sums = smp.tile([P, NBLK], dt.float32, tag="sums")
                    for ci, (start, width, is_b) in enumerate(chunks):
                        ps_s = sp.tile([P, 512 if mixed else CHUNK],
                                       dt.float32, tag="ps_s")
                        for ec in range(EC):
                            nc.tensor.matmul(
                                ps_s[:, 0:width],
                                qt_sb[:, ec, ts(j, P)],
                                kt_sb[:, ec, ds(start, width)],
                                start=(ec == 0),
                                stop=(ec == EC - 1),
                            )
                        if is_b:
                            nc.vector.tensor_add(
                                ps_s[:, 0:width], ps_s[:, 0:width], mask_sb[:]
                            )
                        p_t = wp.tile([P, 512 if mixed else CHUNK],
                                      dt.bfloat16, tag="p")
                        nc.scalar.activation(
                            p_t[:, 0:width], ps_s[:, 0:width], AF.Exp,
                            bias=0.0, scale=SCALE,
                            accum_out=sums[:, ci : ci + 1],
                        )
                        for h in range(width // P):
                            t_idx = start // P + h
                            pt_t = wp.tile([P, P], dt.bfloat16, tag="pt")
                            if dma_t:
                                nc.scalar.dma_start_transpose(pt_t[:], p_t[:, ts(h, P)])
                            else:
                                ps_t = tp.tile([P, P], dt.bfloat16, tag="ps_t")
                                nc.tensor.transpose(ps_t[:], p_t[:, ts(h, P)], ident_sb[:])
                                nc.vector.tensor_copy(pt_t[:], ps_t[:])
                            for en in range(E // 512):
                                nc.tensor.matmul(
                                    ps_out[:, ts(en, 512)],
                                    pt_t[:],
                                    v_sb[:, t_idx, ts(en, 512)],
                                    start=(t_idx == 0),
                                    stop=(is_b and h == width // P - 1),
                                )
                    def _epilogue(j=j, sums=sums, ps_out=ps_out, ncol=len(chunks)):
                        denom = smp.tile([P, 1], dt.float32, tag="denom")
                        nc.vector.reduce_sum(
                            denom[:], sums[:, 0:ncol], axis=mybir.AxisListType.X
                        )
                        recip = smp.tile([P, 1], dt.float32, tag="recip")
                        nc.vector.reciprocal(recip[:], denom[:])
                        out_t = wp.tile([P, E], dt.float32, tag="out_t")
                        for en in range(E // 512):
                            nc.vector.tensor_scalar_mul(
                                out_t[:, ts(en, 512)], ps_out[:, ts(en, 512)], recip[:]
                            )
                            nc.gpsimd.dma_start(
                                out[j][:, ts(en, 512)], out_t[:, ts(en, 512)]
                            )

                    if delay_epi:
                        if pending_epi is not None:
                            pending_epi()
                        pending_epi = _epilogue
                    else:
                        _epilogue()
                if pending_epi is not None:
                    pending_epi()

    nc.compile()
    return nc


def _get_program():
    if "nc" not in _cache:
        _cache["nc"] = _build_program(
            reps=1, pair_kv=True, cc_mode="two", psum_bufs=(3, 3, 1), proj_bufs=8,
            mixed=True,
        )
    return _cache["nc"]


def _make_in_maps(x, Wq, Wk, Wv, pair_kv=True):
    bf16 = ml_dtypes.bfloat16
    wq_b = np.ascontiguousarray(Wq.astype(bf16))
    wk_b = np.ascontiguousarray(Wk.astype(bf16))
    wv_b = np.ascontiguousarray(Wv.astype(bf16))

    # additive causal masks for the boundary chunk, per parity
    r = np.arange(P)[:, None]
    c = np.arange(CHUNK)[None, :]
    masks = [
        np.where(c <= r, 0.0, -1e9).astype(np.float32),        # parity 0
        np.where(c <= r + P, 0.0, -1e9).astype(np.float32),    # parity 1
    ]
    ident = np.eye(P, dtype=bf16)

    in_maps = []
    for core in range(NCORES):
        b, par = core // 2, core % 2
        xt = np.ascontiguousarray(x[b].T.astype(bf16))  # [D, S]
        blocks = [2 * j + par for j in range(NBLK)]
        xt_q = np.ascontiguousarray(
            xt.reshape(D, S // P, P)[:, blocks, :].reshape(D, NBLK * P)
        )
        xt_kv = (
            np.ascontiguousarray(xt[:, par * SH : (par + 1) * SH]) if pair_kv else xt
        )
        in_maps.append(
            {
                "xt_q": xt_q,
                "xt_kv": xt_kv,
                "wq": wq_b,
                "wk": wk_b,
                "wv": wv_b,
                "mask": masks[par],
                "ident": ident,
            }
        )
    return in_maps


def _assemble(results):
    out = np.empty((B, S, E), dtype=np.float32)
    for core in range(NCORES):
        b, par = core // 2, core % 2
        o = results[core]["out"]  # [NBLK, P, E]
        for j in range(NBLK):
            i = 2 * j + par
            out[b, i * P : (i + 1) * P, :] = o[j]
    return out


def run(inputs, trace=False):
    from concourse import bass_utils

    x = np.asarray(inputs["x"], dtype=np.float32)
    Wq = np.asarray(inputs["Wq"], dtype=np.float32)
    Wk = np.asarray(inputs["Wk"], dtype=np.float32)
    Wv = np.asarray(inputs["Wv"], dtype=np.float32)

    nc = _get_program()
    in_maps = _make_in_maps(x, Wq, Wk, Wv)
    res = bass_utils.run_bass_kernel_spmd(
        nc, in_maps, core_ids=list(range(NCORES)), trace=trace
    )
    return _assemble(res.results), res


def kernel(**inputs):
    out, _ = run(inputs, trace=False)
    return out


# revision 30
# speedup vs baseline: 2.0730x; 1.2640x over previous
"""Causal single-head attention (B=4, S=2048, D=1024, E=1024) on 8 TRN2 cores.

Sharding: 2 cores per batch. Within a batch, core parity p takes query
blocks {2j+p : j=0..7} (128 rows each) — interleaving balances the causal
triangle so both cores do identical work SHAPES (SPMD-clean) and identical
work AMOUNTS.

K/V projections are pair-split: each core projects K^T/V for its half of
the keys (parity 0 → keys [0,1024), parity 1 → [1024,2048)), then the two
cores of a batch exchange halves with two pipelined 2-core AllGathers
(K^T's issued right after the K projection so it overlaps the V and Q
projections; V's overlaps the Q projection).

All matmuls run on the PE in bf16 (fp32 PSUM accumulate). Host pre-transposes
x to [d, s] layout and pre-casts to bf16 so the kernel needs no on-chip
transposes of x. The causal boundary mask is per-core DATA (additive -1e9
tile applied to the last 256-key chunk of every query block).
"""

import sys

if "/opt/trn_rl_repo" not in sys.path:
    sys.path.insert(0, "/opt/trn_rl_repo")

import numpy as np
import ml_dtypes

B, S, D, E = 4, 2048, 1024, 1024
NCORES = 8
NBLK = 8          # query blocks per core (128 rows each)
CHUNK = 256       # key chunk width for scores
P = 128
SH = S // 2       # keys projected per core
SCALE = 1.0 / 32.0  # 1/sqrt(E)

_cache = {}


class _nullpool:
    def __enter__(self):
        return None

    def __exit__(self, *a):
        return False



def _build_program(reps=1, pair_kv=True, cc_mode="one", psum_bufs=(2, 2, 2), proj_bufs=4, desc=False, wp_bufs=4, dma_t=False, mixed=False, delay_epi=False):
    import concourse.bass as bass
    import concourse.tile as tile
    from concourse import bacc, mybir
    from concourse.bass import ts, ds
    from contextlib import ExitStack

    dt = mybir.dt
    AF = mybir.ActivationFunctionType

    nc = bacc.Bacc(
        "TRN2", target_bir_lowering=False, debug=False, enable_asserts=False,
        num_devices=NCORES,
    )

    kvw = SH if pair_kv else S
    xt_q = nc.dram_tensor("xt_q", [D, NBLK * P], dt.bfloat16, kind="ExternalInput").ap()
    xt_kv = nc.dram_tensor("xt_kv", [D, kvw], dt.bfloat16, kind="ExternalInput").ap()
    wq = nc.dram_tensor("wq", [D, E], dt.bfloat16, kind="ExternalInput").ap()
    wk = nc.dram_tensor("wk", [D, E], dt.bfloat16, kind="ExternalInput").ap()
    wv = nc.dram_tensor("wv", [D, E], dt.bfloat16, kind="ExternalInput").ap()
    maskd = nc.dram_tensor("mask", [P, CHUNK], dt.float32, kind="ExternalInput").ap()
    identd = nc.dram_tensor("ident", [P, P], dt.bfloat16, kind="ExternalInput").ap()
    out = nc.dram_tensor("out", [NBLK, P, E], dt.float32, kind="ExternalOutput").ap()

    if pair_kv:
        # packed [K^T_half (E x SH) ; V_half (SH x E)] exchange buffers
        cc_in = nc.dram_tensor("cc_in", [2, E, SH], dt.bfloat16).ap()
        cc_out = nc.dram_tensor("cc_out", [4, E, SH], dt.bfloat16).ap()
        cc_outk = nc.dram_tensor("cc_outk", [2, E, SH], dt.bfloat16).ap()
        cc_outv = nc.dram_tensor("cc_outv", [2, SH, E], dt.bfloat16).ap()

    DC = D // P   # 8 contraction chunks
    EC = E // P   # 8 e chunks
    TC = S // P   # 16 key chunks of 128

    with tile.TileContext(nc) as tc, ExitStack() as ctx:
        consts = ctx.enter_context(tc.tile_pool(name="consts", bufs=1))

        wq_sb = consts.tile([P, DC, E], dt.bfloat16, tag="wq")
        wk_sb = consts.tile([P, DC, E], dt.bfloat16, tag="wk")
        wv_sb = consts.tile([P, DC, E], dt.bfloat16, tag="wv")
        xq_sb = consts.tile([P, DC, NBLK * P], dt.bfloat16, tag="xq")
        xkv_sb = consts.tile([P, DC, kvw], dt.bfloat16, tag="xkv")
        qt_sb = consts.tile([P, EC, NBLK * P], dt.bfloat16, tag="qt")
        kt_sb = consts.tile([P, EC, S], dt.bfloat16, tag="kt")
        v_sb = consts.tile([P, TC, E], dt.bfloat16, tag="v")
        mask_sb = consts.tile([P, CHUNK], dt.float32, tag="mask")
        ident_sb = consts.tile([P, P], dt.bfloat16, tag="ident")

        nc.sync.dma_start(wk_sb[:, 0, 0:P], wk[0:P, 0:P])
        nc.sync.dma_start(xkv_sb[:, 0, 0:512], xt_kv[0:P, 0:512])
        nc.sync.dma_start(wk_sb[:, 0, P:E], wk[0:P, P:E])
        nc.sync.dma_start(xkv_sb[:, 0, 512:kvw], xt_kv[0:P, 512:kvw])
        nc.sync.dma_start(mask_sb[:], maskd[:])
        nc.sync.dma_start(ident_sb[:], identd[:])
        for dc in range(1, DC):
            nc.sync.dma_start(wk_sb[:, dc, :], wk[dc * P : (dc + 1) * P, :])
            nc.sync.dma_start(xkv_sb[:, dc, :], xt_kv[dc * P : (dc + 1) * P, :])
        for dc in range(DC):
            nc.sync.dma_start(wv_sb[:, dc, :], wv[dc * P : (dc + 1) * P, :])
        for dc in range(DC):
            nc.sync.dma_start(wq_sb[:, dc, :], wq[dc * P : (dc + 1) * P, :])
            nc.sync.dma_start(xq_sb[:, dc, :], xt_q[dc * P : (dc + 1) * P, :])

        for _rep in range(reps):
            # ---- Phase 1: projections (PE, bf16, accumulate over d in PSUM) ----
            with (
                tc.tile_pool(name="proj_ps", bufs=proj_bufs, space="PSUM") as pp,
                tc.tile_pool(name="stage", bufs=8) as stg,
            ):
                if pair_kv:
                    # K^T half [e, t_local] = Wk^T @ x_kv^T  -> cc_in[0]
                    for ec in range(EC):
                        for tn in range(SH // 512):
                            ps = pp.tile([P, 512], dt.float32, tag="proj")
                            for dc in range(DC):
                                nc.tensor.matmul(
                                    ps[:],
                                    wk_sb[:, dc, ts(ec, P)],
                                    xkv_sb[:, dc, ts(tn, 512)],
                                    start=(dc == 0),
                                    stop=(dc == DC - 1),
                                )
                            st = stg.tile([P, 512], dt.bfloat16, tag="st")
                            nc.any.tensor_copy(st[:], ps[:])
                            nc.gpsimd.dma_start(
                                cc_in[0, ec * P : (ec + 1) * P, ts(tn, 512)], st[:]
                            )
                    if cc_mode == "two":
                        nc.gpsimd.collective_compute(
                            "AllGather",
                            mybir.AluOpType.bypass,
                            replica_groups=[[0, 1], [2, 3], [4, 5], [6, 7]],
                            ins=[cc_in[0]],
                            outs=[cc_outk[:]],
                        )
                    # V half [t_local, e] = x_kv @ Wv  -> cc_in[1]
                    for tcc in range(SH // P):
                        for en in range(E // 512):
                            ps = pp.tile([P, 512], dt.float32, tag="proj")
                            for dc in range(DC):
                                nc.tensor.matmul(
                                    ps[:],
                                    xkv_sb[:, dc, ts(tcc, P)],
                                    wv_sb[:, dc, ts(en, 512)],
                                    start=(dc == 0),
                                    stop=(dc == DC - 1),
                                )
                            st = stg.tile([P, 512], dt.bfloat16, tag="st")
                            nc.any.tensor_copy(st[:], ps[:])
                            nc.gpsimd.dma_start(
                                cc_in[1, tcc * P : (tcc + 1) * P, ts(en, 512)], st[:]
                            )
                    if cc_mode == "two":
                        nc.gpsimd.collective_compute(
                            "AllGather",
                            mybir.AluOpType.bypass,
                            replica_groups=[[0, 1], [2, 3], [4, 5], [6, 7]],
                            ins=[cc_in[1]],
                            outs=[cc_outv[:]],
                        )
                    if cc_mode == "one":
                        nc.gpsimd.collective_compute(
                            "AllGather",
                            mybir.AluOpType.bypass,
                            replica_groups=[[0, 1], [2, 3], [4, 5], [6, 7]],
                            ins=[cc_in[:]],
                            outs=[cc_out[:]],
                        )
                    elif cc_mode == "fake":
                        nc.sync.dma_start(cc_out[0], cc_in[0])
                        nc.sync.dma_start(cc_out[1], cc_in[1])
                        nc.sync.dma_start(cc_out[2], cc_in[0])
                        nc.sync.dma_start(cc_out[3], cc_in[1])
                else:
                    for ec in range(EC):
                        for tn in range(S // 512):
                            ps = pp.tile([P, 512], dt.float32, tag="proj")
                            for dc in range(DC):
                                nc.tensor.matmul(
                                    ps[:],
                                    wk_sb[:, dc, ts(ec, P)],
                                    xkv_sb[:, dc, ts(tn, 512)],
                                    start=(dc == 0),
                                    stop=(dc == DC - 1),
                                )
                            nc.any.tensor_copy(kt_sb[:, ec, ts(tn, 512)], ps[:])
                    for tcc in range(TC):
                        for en in range(E // 512):
                            ps = pp.tile([P, 512], dt.float32, tag="proj")
                            for dc in range(DC):
                                nc.tensor.matmul(
                                    ps[:],
                                    xkv_sb[:, dc, ts(tcc, P)],
                                    wv_sb[:, dc, ts(en, 512)],
                                    start=(dc == 0),
                                    stop=(dc == DC - 1),
                                )
                            nc.any.tensor_copy(v_sb[:, tcc, ts(en, 512)], ps[:])

                # Q^T [e, q] = Wq^T @ x_q^T  (overlaps the collective)
                for ec in range(EC):
                    for qn in range(NBLK * P // 512):
                        ps = pp.tile([P, 512], dt.float32, tag="proj")
                        for dc in range(DC):
                            nc.tensor.matmul(
                                ps[:],
                                wq_sb[:, dc, ts(ec, P)],
                                xq_sb[:, dc, ts(qn, 512)],
                                start=(dc == 0),
                                stop=(dc == DC - 1),
                            )
                        nc.any.tensor_copy(qt_sb[:, ec, ts(qn, 512)], ps[:])

                if pair_kv:
                    # unpack gathered halves into full K^T / V in SBUF
                    for r in range(2):
                        ksrc = cc_outk[r] if cc_mode == "two" else cc_out[2 * r]
                        vsrc = cc_outv[r] if cc_mode == "two" else cc_out[2 * r + 1]
                        for ec in range(EC):
                            nc.gpsimd.dma_start(
                                kt_sb[:, ec, ds(r * SH, SH)],
                                ksrc[ec * P : (ec + 1) * P, :],
                            )
                        for tcl in range(SH // P):
                            nc.gpsimd.dma_start(
                                v_sb[:, r * (SH // P) + tcl, :],
                                vsrc[tcl * P : (tcl + 1) * P, :],
                            )

            # ---- Phase 2: attention ----
            with (
                tc.tile_pool(name="score_ps", bufs=psum_bufs[0], space="PSUM") as sp,
                (tc.tile_pool(name="pt_ps", bufs=psum_bufs[1], space="PSUM")
                 if not dma_t else _nullpool()) as tp,
                tc.tile_pool(name="out_ps", bufs=psum_bufs[2], space="PSUM") as op,
                tc.tile_pool(name="work", bufs=wp_bufs) as wp,
                tc.tile_pool(name="small", bufs=4) as smp,
            ):
                pending_epi = None
                for j in (reversed(range(NBLK)) if desc else range(NBLK)):
                    # chunk plan: (start_key, width, is_boundary); same trip
                    # structure on every core (depends only on slot j)
                    if mixed:
                        chunks = []
                        off = 0
                        for _ in range(j // 2):
                            chunks.append((off, 512, False)); off += 512
                        if j % 2:
                            chunks.append((off, 256, False)); off += 256
                        chunks.append((off, CHUNK, True))
                    else:
                        chunks = [
                            (c * CHUNK, CHUNK, c == j) for c in range(j + 1)
                        ]
                    ps_out = op.tile([P, E], dt.float32, tag="ps_out")
                    sums = smp.tile([P, NBLK], dt.float32, tag="sums")
                    for ci, (start, width, is_b) in enumerate(chunks):
                        ps_s = sp.tile([P, 512 if mixed else CHUNK],
                                       dt.float32, tag="ps_s")
                        for ec in range(EC):
                            nc.tensor.matmul(
                                ps_s[:, 0:width],
                                qt_sb[:, ec, ts(j, P)],
                                kt_sb[:, ec, ds(start, width)],
                                start=(ec == 0),
                                stop=(ec == EC - 1),
                            )
                        if is_b:
                            nc.vector.tensor_add(
                                ps_s[:, 0:width], ps_s[:, 0:width], mask_sb[:]
                            )
                        p_t = wp.tile([P, 512 if mixed else CHUNK],
                                      dt.bfloat16, tag="p")
                        nc.scalar.activation(
                            p_t[:, 0:width], ps_s[:, 0:width], AF.Exp,
                            bias=0.0, scale=SCALE,
                            accum_out=sums[:, ci : ci + 1],
                        )
                        for h in range(width // P):
                            t_idx = start // P + h
                            pt_t = wp.tile([P, P], dt.bfloat16, tag="pt")
                            if dma_t:
                                nc.scalar.dma_start_transpose(pt_t[:], p_t[:, ts(h, P)])
                            else:
                                ps_t = tp.tile([P, P], dt.bfloat16, tag="ps_t")
                                nc.tensor.transpose(ps_t[:], p_t[:, ts(h, P)], ident_sb[:])
                                nc.vector.tensor_copy(pt_t[:], ps_t[:])
                            for en in range(E // 512):
                                nc.tensor.matmul(
                                    ps_out[:, ts(en, 512)],
                                    pt_t[:],
                                    v_sb[:, t_idx, ts(en, 512)],
                                    start=(t_idx == 0),
                                    stop=(is_b and h == width // P - 1),
                                )
                    def _epilogue(j=j, sums=sums, ps_out=ps_out, ncol=len(chunks)):
                        denom = smp.tile([P, 1], dt.float32, tag="denom")
                        nc.vector.reduce_sum(
                            denom[:], sums[:, 0:ncol], axis=mybir.AxisListType.X
                        )
                        recip = smp.tile([P, 1], dt.float32, tag="recip")
                        nc.vector.reciprocal(recip[:], denom[:])
                        out_t = wp.tile([P, E], dt.float32, tag="out_t")
                        for en in range(E // 512):
                            nc.vector.tensor_scalar_mul(
                                out_t[:, ts(en, 512)], ps_out[:, ts(en, 512)], recip[:]
                            )
                            nc.gpsimd.dma_start(
                                out[j][:, ts(en, 512)], out_t[:, ts(en, 512)]
                            )

                    if delay_epi:
                        if pending_epi is not None:
                            pending_epi()
                        pending_epi = _epilogue
                    else:
                        _epilogue()
                if pending_epi is not None:
                    pending_epi()

    nc.compile()
    return nc


def _get_program():
    if "nc" not in _cache:
        _cache["nc"] = _build_program(
            reps=1, pair_kv=True, cc_mode="two", psum_bufs=(3, 3, 1), proj_bufs=8,
            mixed=True,
        )
    return _cache["nc"]


def _make_in_maps(x, Wq, Wk, Wv, pair_kv=True):
    bf16 = ml_dtypes.bfloat16
    wq_b = np.ascontiguousarray(Wq.astype(bf16))
    wk_b = np.ascontiguousarray(Wk.astype(bf16))
    wv_b = np.ascontiguousarray(Wv.astype(bf16))

    # additive causal masks for the boundary chunk, per parity
    r = np.arange(P)[:, None]
    c = np.arange(CHUNK)[None, :]
    masks = [
        np.where(c <= r, 0.0, -1e9).astype(np.float32),        # parity 0
        np.where(c <= r + P, 0.0, -1e9).astype(np.float32),    # parity 1
    ]
    ident = np.eye(P, dtype=bf16)

    in_maps = []
    for core in range(NCORES):
        b, par = core // 2, core % 2
        xt = np.ascontiguousarray(x[b].T.astype(bf16))  # [D, S]
        blocks = [2 * j + par for j in range(NBLK)]
        xt_q = np.ascontiguousarray(
            xt.reshape(D, S // P, P)[:, blocks, :].reshape(D, NBLK * P)
        )
        xt_kv = (
            np.ascontiguousarray(xt[:, par * SH : (par + 1) * SH]) if pair_kv else xt
        )
        in_maps.append(
            {
                "xt_q": xt_q,
                "xt_kv": xt_kv,
                "wq": wq_b,
                "wk": wk_b,
                "wv": wv_b,
                "mask": masks[par],
                "ident": ident,
            }
        )
    return in_maps


def _assemble(results):
    out = np.empty((B, S, E), dtype=np.float32)
    for core in range(NCORES):
        b, par = core // 2, core % 2
        o = results[core]["out"]  # [NBLK, P, E]
        for j in range(NBLK):
            i = 2 * j + par
            out[b, i * P : (i + 1) * P, :] = o[j]
    return out


def run(inputs, trace=False):
    from concourse import bass_utils

    x = np.asarray(inputs["x"], dtype=np.float32)
    Wq = np.asarray(inputs["Wq"], dtype=np.float32)
    Wk = np.asarray(inputs["Wk"], dtype=np.float32)
    Wv = np.asarray(inputs["Wv"], dtype=np.float32)

    nc = _get_program()
    in_maps = _make_in_maps(x, Wq, Wk, Wv)
    res = bass_utils.run_bass_kernel_spmd(
        nc, in_maps, core_ids=list(range(NCORES)), trace=trace
    )
    return _assemble(res.results), res


def kernel(**inputs):
    out, _ = run(inputs, trace=False)
    return out
